# revision 10
# baseline (speedup 1.0000x reference)
"""L1-loss kernel for Trainium2: mean over rows of sum(|out - target|).

Data-parallel over 8 NeuronCores: each core streams its row-shard of
`out` and `target` from HBM and produces per-partition partial sums of
|out - target|; the host sums the partials and divides by the global
row count.

Per core the shard is repacked host-side into [NT, 128, 2*FREE] tiles
whose partition rows hold the `out` chunk followed by the `target`
chunk. One DMA then feeds both operands of the subtract, which halves
the DMA count and keeps each compute instruction to a single upstream
semaphore. Per tile: DVE subtract into a scratch tile, then ACT Abs
with free-dim accumulation into a [128, NT] accumulator column — the
two compute engines each make one pass, well under the ~360 GB/s DMA
stream that bounds the kernel (memory-roofline workload).
"""

from contextlib import ExitStack

import numpy as np

import concourse.bass as bass
import concourse.bacc as bacc
import concourse.tile as tile
from concourse import mybir
from concourse.bass_utils import run_bass_kernel_spmd

N_VEH = 8388608
N_FEAT = 8
N_CORES = 8
ROWS_PER_CORE = N_VEH // N_CORES            # 1048576
ELEMS_PER_CORE = ROWS_PER_CORE * N_FEAT     # 8388608
P = 128
FREE = 2048
NT = ELEMS_PER_CORE // (P * FREE)           # 32 tiles; fused tile = [128, 4096] f32 (2 MiB)


def _build_nc() -> bass.Bass:
    # Bacc (not raw Bass): its compile() pass allocates registers and splits
    # multi-sem waits into EventSemaphore instructions — TRN2 instructions
    # fit only one wait.
    nc = bacc.Bacc()
    ot_ext = nc.declare_dram_parameter(
        "ot", [NT, P, 2 * FREE], mybir.dt.float32, isOutput=False
    )
    partials = nc.declare_dram_parameter(
        "partials", [P, NT], mybir.dt.float32, isOutput=True
    )

    with tile.TileContext(nc) as tc, ExitStack() as ctx:
        x_pool = ctx.enter_context(tc.tile_pool(name="x", bufs=6))
        d_pool = ctx.enter_context(tc.tile_pool(name="d", bufs=2))
        acc_pool = ctx.enter_context(tc.tile_pool(name="acc", bufs=1))
        acc = acc_pool.tile([P, NT], mybir.dt.float32)
        for i in range(NT):
            x = x_pool.tile([P, 2 * FREE], mybir.dt.float32)
            nc.sync.dma_start(x[:], ot_ext[i])
            d = d_pool.tile([P, FREE], mybir.dt.float32)
            nc.vector.tensor_tensor(
                out=d[:], in0=x[:, :FREE], in1=x[:, FREE:],
                op=mybir.AluOpType.subtract,
            )
            nc.scalar.activation(
                out=d[:], in_=d[:],
                func=mybir.ActivationFunctionType.Abs,
                accum_out=acc[:, i : i + 1],
            )
        nc.sync.dma_start(partials[:], acc[:])
    # The PJRT exec path serializes the module as-is; finalize() here runs
    # Bacc.compile() (register allocation + the wait-splitting pass).
    nc.finalize()
    return nc


def _pack(out: np.ndarray, target: np.ndarray) -> list[dict[str, np.ndarray]]:
    """Interleave out/target per partition row: core shard -> [NT, P, 2*FREE]."""
    in_maps = []
    for c in range(N_CORES):
        sl = slice(c * ROWS_PER_CORE, (c + 1) * ROWS_PER_CORE)
        ot = np.empty((NT, P, 2 * FREE), dtype=np.float32)
        ot[:, :, :FREE] = out[sl].reshape(NT, P, FREE)
        ot[:, :, FREE:] = target[sl].reshape(NT, P, FREE)
        in_maps.append({"ot": ot})
    return in_maps


def _run(nc: bass.Bass, out: np.ndarray, target: np.ndarray, **kwargs):
    return run_bass_kernel_spmd(nc, _pack(out, target), list(range(N_CORES)), **kwargs)


def kernel(out: np.ndarray, target: np.ndarray, x: np.ndarray | None = None) -> np.ndarray:
    out = np.ascontiguousarray(np.asarray(out, dtype=np.float32))
    target = np.ascontiguousarray(np.asarray(target, dtype=np.float32))
    res = _run(_build_nc(), out, target)
    total = sum(r["partials"].astype(np.float64).sum() for r in res.results)
    return np.asarray(total / N_VEH, dtype=np.float32)


# revision 11
# speedup vs baseline: 1.0092x; 1.0092x over previous
"""L1-loss kernel for Trainium2: mean over rows of sum(|out - target|).

Data-parallel over 8 NeuronCores: each core streams its row-shard of
`out` and `target` from HBM and produces per-partition partial sums of
|out - target|; the host sums the partials and divides by the global
row count.

Per core the shard is repacked host-side into [128, 2*FREE] tiles whose
partition rows hold the `out` chunk followed by the `target` chunk. One
DMA then feeds both operands of the subtract, which halves the DMA
count and keeps each compute instruction to a single upstream
semaphore. Per tile: DVE subtract into a scratch tile, then ACT Abs
with free-dim accumulation into an accumulator column — the two compute
engines each make one pass, fully hidden under the ~360 GB/s DMA stream
that bounds this memory-roofline workload.

Tail trimming: the last tile is packed as two half-tiles so its compute
pipelines with its loads, and the accumulator columns for the first
NT-1 tiles are flushed to DRAM early — only the two tail columns remain
on the critical path after the final load.
"""

from contextlib import ExitStack

import numpy as np

import concourse.bass as bass
import concourse.bacc as bacc
import concourse.tile as tile
from concourse import mybir
from concourse.bass_utils import run_bass_kernel_spmd

N_VEH = 8388608
N_FEAT = 8
N_CORES = 8
ROWS_PER_CORE = N_VEH // N_CORES            # 1048576
ELEMS_PER_CORE = ROWS_PER_CORE * N_FEAT     # 8388608
P = 128
FREE = 2048
NT = ELEMS_PER_CORE // (P * FREE)           # 32 tiles; fused tile = [128, 4096] f32 (2 MiB)
H = FREE // 2                               # tail half-tile free size
NCOL = NT + 1                               # NT-1 full columns + 2 tail columns


def _build_nc() -> bass.Bass:
    # Bacc (not raw Bass): its compile() pass allocates registers and splits
    # multi-sem waits into EventSemaphore instructions — TRN2 instructions
    # fit only one wait. The PJRT exec path serializes the module as-is, so
    # finalize() must be called here.
    nc = bacc.Bacc()
    ot_ext = nc.declare_dram_parameter(
        "ot", [NT - 1, P, 2 * FREE], mybir.dt.float32, isOutput=False
    )
    ott_ext = nc.declare_dram_parameter(
        "ott", [2, P, 2 * H], mybir.dt.float32, isOutput=False
    )
    partials = nc.declare_dram_parameter(
        "partials", [P, NCOL], mybir.dt.float32, isOutput=True
    )

    with tile.TileContext(nc) as tc, ExitStack() as ctx:
        x_pool = ctx.enter_context(tc.tile_pool(name="x", bufs=6))
        d_pool = ctx.enter_context(tc.tile_pool(name="d", bufs=2))
        acc_pool = ctx.enter_context(tc.tile_pool(name="acc", bufs=1))
        acc = acc_pool.tile([P, NCOL], mybir.dt.float32)
        for i in range(NT - 1):
            x = x_pool.tile([P, 2 * FREE], mybir.dt.float32)
            nc.sync.dma_start(x[:], ot_ext[i])
            d = d_pool.tile([P, FREE], mybir.dt.float32)
            nc.vector.tensor_tensor(
                out=d[:], in0=x[:, :FREE], in1=x[:, FREE:],
                op=mybir.AluOpType.subtract,
            )
            nc.scalar.activation(
                out=d[:], in_=d[:],
                func=mybir.ActivationFunctionType.Abs,
                accum_out=acc[:, i : i + 1],
            )
        nc.sync.dma_start(partials[:, : NT - 1], acc[:, : NT - 1])
        for k in range(2):
            xk = x_pool.tile([P, 2 * H], mybir.dt.float32, tag="xtail")
            nc.sync.dma_start(xk[:], ott_ext[k])
            dk = d_pool.tile([P, H], mybir.dt.float32, tag="dtail")
            nc.vector.tensor_tensor(
                out=dk[:], in0=xk[:, :H], in1=xk[:, H:],
                op=mybir.AluOpType.subtract,
            )
            nc.scalar.activation(
                out=dk[:], in_=dk[:],
                func=mybir.ActivationFunctionType.Abs,
                accum_out=acc[:, NT - 1 + k : NT + k],
            )
        nc.sync.dma_start(partials[:, NT - 1 :], acc[:, NT - 1 :])
    nc.finalize()
    return nc


def _pack(out: np.ndarray, target: np.ndarray) -> list[dict[str, np.ndarray]]:
    """Interleave out/target per partition row; last tile as two half-tiles."""
    in_maps = []
    for c in range(N_CORES):
        sl = slice(c * ROWS_PER_CORE, (c + 1) * ROWS_PER_CORE)
        o = out[sl].reshape(NT, P, FREE)
        t = target[sl].reshape(NT, P, FREE)
        ot = np.empty((NT - 1, P, 2 * FREE), dtype=np.float32)
        ot[:, :, :FREE] = o[: NT - 1]
        ot[:, :, FREE:] = t[: NT - 1]
        ott = np.empty((2, P, 2 * H), dtype=np.float32)
        for k in range(2):
            ott[k, :, :H] = o[NT - 1, :, k * H : (k + 1) * H]
            ott[k, :, H:] = t[NT - 1, :, k * H : (k + 1) * H]
        in_maps.append({"ot": ot, "ott": ott})
    return in_maps


def _run(nc: bass.Bass, out: np.ndarray, target: np.ndarray, **kwargs):
    return run_bass_kernel_spmd(nc, _pack(out, target), list(range(N_CORES)), **kwargs)


def kernel(out: np.ndarray, target: np.ndarray, x: np.ndarray | None = None) -> np.ndarray:
    out = np.ascontiguousarray(np.asarray(out, dtype=np.float32))
    target = np.ascontiguousarray(np.asarray(target, dtype=np.float32))
    res = _run(_build_nc(), out, target)
    total = sum(r["partials"].astype(np.float64).sum() for r in res.results)
    return np.asarray(total / N_VEH, dtype=np.float32)


# revision 12
# speedup vs baseline: 1.0150x; 1.0058x over previous
"""L1-loss kernel for Trainium2: mean over rows of sum(|out - target|).

Data-parallel over 8 NeuronCores: each core streams its row-shard of
`out` and `target` from HBM and produces per-partition partial sums of
|out - target|; the host sums the partials and divides by the global
row count.

Per core the shard is repacked host-side into [128, 2*FREE] tiles whose
partition rows hold the `out` chunk followed by the `target` chunk. One
DMA then feeds both operands of the subtract, which halves the DMA
count and keeps each compute instruction to a single upstream
semaphore. Per tile: DVE subtract into a scratch tile, then ACT Abs
with free-dim accumulation into an accumulator column — the two compute
engines each make one pass, fully hidden under the ~360 GB/s DMA stream
that bounds this memory-roofline workload.

Tail trimming: the last tile is packed as two half-tiles so its compute
pipelines with its loads, and the accumulator columns for the first
NT-1 tiles are flushed to DRAM early — only the two tail columns remain
on the critical path after the final load.
"""

from contextlib import ExitStack

import numpy as np

import concourse.bass as bass
import concourse.bacc as bacc
import concourse.tile as tile
from concourse import mybir
from concourse.bass_utils import run_bass_kernel_spmd

N_VEH = 8388608
N_FEAT = 8
N_CORES = 8
ROWS_PER_CORE = N_VEH // N_CORES            # 1048576
ELEMS_PER_CORE = ROWS_PER_CORE * N_FEAT     # 8388608
P = 128
FREE = 2048
NT = ELEMS_PER_CORE // (P * FREE)           # 32 tiles; fused tile = [128, 4096] f32 (2 MiB)
TAIL_CHUNKS = 4                             # last tile split for tail overlap
H = FREE // TAIL_CHUNKS                     # tail chunk free size
NCOL = NT - 1 + TAIL_CHUNKS                 # NT-1 full columns + tail columns


def _build_nc() -> bass.Bass:
    # Bacc (not raw Bass): its compile() pass allocates registers and splits
    # multi-sem waits into EventSemaphore instructions — TRN2 instructions
    # fit only one wait. The PJRT exec path serializes the module as-is, so
    # finalize() must be called here.
    nc = bacc.Bacc()
    ot_ext = nc.declare_dram_parameter(
        "ot", [NT - 1, P, 2 * FREE], mybir.dt.float32, isOutput=False
    )
    ott_ext = nc.declare_dram_parameter(
        "ott", [TAIL_CHUNKS, P, 2 * H], mybir.dt.float32, isOutput=False
    )
    partials = nc.declare_dram_parameter(
        "partials", [P, NCOL], mybir.dt.float32, isOutput=True
    )

    with tile.TileContext(nc) as tc, ExitStack() as ctx:
        x_pool = ctx.enter_context(tc.tile_pool(name="x", bufs=6))
        d_pool = ctx.enter_context(tc.tile_pool(name="d", bufs=2))
        acc_pool = ctx.enter_context(tc.tile_pool(name="acc", bufs=1))
        acc = acc_pool.tile([P, NCOL], mybir.dt.float32)
        for i in range(NT - 1):
            x = x_pool.tile([P, 2 * FREE], mybir.dt.float32)
            nc.sync.dma_start(x[:], ot_ext[i])
            d = d_pool.tile([P, FREE], mybir.dt.float32)
            nc.vector.tensor_tensor(
                out=d[:], in0=x[:, :FREE], in1=x[:, FREE:],
                op=mybir.AluOpType.subtract,
            )
            nc.scalar.activation(
                out=d[:], in_=d[:],
                func=mybir.ActivationFunctionType.Abs,
                accum_out=acc[:, i : i + 1],
            )
        xs = []
        for k in range(TAIL_CHUNKS):
            xk = x_pool.tile([P, 2 * H], mybir.dt.float32, tag="xtail")
            nc.sync.dma_start(xk[:], ott_ext[k])
            xs.append(xk)
        nc.sync.dma_start(partials[:, : NT - 1], acc[:, : NT - 1])
        for k in range(TAIL_CHUNKS):
            dk = d_pool.tile([P, H], mybir.dt.float32, tag="dtail")
            nc.vector.tensor_tensor(
                out=dk[:], in0=xs[k][:, :H], in1=xs[k][:, H:],
                op=mybir.AluOpType.subtract,
            )
            nc.scalar.activation(
                out=dk[:], in_=dk[:],
                func=mybir.ActivationFunctionType.Abs,
                accum_out=acc[:, NT - 1 + k : NT + k],
            )
        nc.sync.dma_start(partials[:, NT - 1 :], acc[:, NT - 1 :])
    nc.finalize()
    return nc


def _pack(out: np.ndarray, target: np.ndarray) -> list[dict[str, np.ndarray]]:
    """Interleave out/target per partition row; last tile as two half-tiles."""
    in_maps = []
    for c in range(N_CORES):
        sl = slice(c * ROWS_PER_CORE, (c + 1) * ROWS_PER_CORE)
        o = out[sl].reshape(NT, P, FREE)
        t = target[sl].reshape(NT, P, FREE)
        ot = np.empty((NT - 1, P, 2 * FREE), dtype=np.float32)
        ot[:, :, :FREE] = o[: NT - 1]
        ot[:, :, FREE:] = t[: NT - 1]
        ott = np.empty((TAIL_CHUNKS, P, 2 * H), dtype=np.float32)
        for k in range(TAIL_CHUNKS):
            ott[k, :, :H] = o[NT - 1, :, k * H : (k + 1) * H]
            ott[k, :, H:] = t[NT - 1, :, k * H : (k + 1) * H]
        in_maps.append({"ot": ot, "ott": ott})
    return in_maps


def _run(nc: bass.Bass, out: np.ndarray, target: np.ndarray, **kwargs):
    return run_bass_kernel_spmd(nc, _pack(out, target), list(range(N_CORES)), **kwargs)


def kernel(out: np.ndarray, target: np.ndarray, x: np.ndarray | None = None) -> np.ndarray:
    out = np.ascontiguousarray(np.asarray(out, dtype=np.float32))
    target = np.ascontiguousarray(np.asarray(target, dtype=np.float32))
    res = _run(_build_nc(), out, target)
    total = sum(r["partials"].astype(np.float64).sum() for r in res.results)
    return np.asarray(total / N_VEH, dtype=np.float32)
